# revision 10
# baseline (speedup 1.0000x reference)
"""CapsuleLinear (k-means routing) Trainium2 kernel.

Math: priors[b,o,i,j] = sum_l w[o,j,l] x[b,i,l]; 3 rounds of k-means routing
over in_capsules, squash=False.

priors is never materialized.  With G_o = W_o^T W_o (64x64 per out-capsule,
computed on-device once):

    u_0[b,l]   = sum_i x[b,i,l]                  (scale of u is irrelevant)
    per iter:  p = G_o u;  q = u.p = ||W u||^2
               rq = exp(-0.5 ln q)  (= 1/||W u||, one ACT table: Ln/Exp/Copy)
               v = p * rq           (v = W^T out_normalized)
               logits[i,o] = sum_l x[b,i,l] v[o,l]
               e = exp(logits)      (softmax Z cancels in v)
               u[o,l] = sum_i e[i,o] x[b,i,l];  Z[o] = sum_i e[i,o]
    output:    out[b,o,:] = W_o u_3[o,:] / Z_3[o]

Sharding: data-parallel over batch, 4 samples/core x 8 cores, weight
replicated, no collectives.  Host passes pre-transposed bf16 layouts
(xb with a ones column, xT, w as (j,o,l) and (l,o,j)) so the device does
no dtype conversion or weight transposition.
"""

import sys

if "/opt/trn_rl_repo" not in sys.path:
    sys.path.insert(0, "/opt/trn_rl_repo")

from contextlib import ExitStack

import ml_dtypes
import numpy as np

import concourse.bacc as bacc
import concourse.bass as bass
import concourse.bass_utils as bass_utils
import concourse.mybir as mybir
import concourse.tile as tile
from concourse.masks import make_identity

BF = mybir.dt.bfloat16
F32 = mybir.dt.float32
AF = mybir.ActivationFunctionType
ALU = mybir.AluOpType

B_GLOBAL = 32
N_CORES = 8
B = B_GLOBAL // N_CORES  # 4 samples per core
O = 128   # out_capsules
I = 512   # in_capsules
J = 64    # out_length
L = 64    # in_length
C = 4     # i-chunks of 128
NITER = 3
WCH = 8   # w DMA chunks
OCH = O // WCH


def _body(ctx: ExitStack, tc: "tile.TileContext", xb_d, xT_d, wj_d, wt_d,
          out_d, probe=None):
    nc = tc.nc

    const_pool = ctx.enter_context(tc.tile_pool(name="const", bufs=1))
    big = ctx.enter_context(tc.tile_pool(name="big", bufs=1))
    sb = ctx.enter_context(tc.tile_pool(name="sb", bufs=2))
    sbE = ctx.enter_context(tc.tile_pool(name="sbE", bufs=3))
    psP = ctx.enter_context(tc.tile_pool(name="psP", bufs=2, space="PSUM"))
    psL = ctx.enter_context(tc.tile_pool(name="psL", bufs=2, space="PSUM"))
    psU = ctx.enter_context(tc.tile_pool(name="psU", bufs=1, space="PSUM"))
    psQ = ctx.enter_context(tc.tile_pool(name="psQ", bufs=1, space="PSUM"))
    psR = ctx.enter_context(tc.tile_pool(name="psR", bufs=1, space="PSUM"))
    psB = ctx.enter_context(tc.tile_pool(name="psB", bufs=1, space="PSUM"))

    # ---- input DMAs (w chunks first: G is the long head pole) ----
    wj_tiles = []
    for k in range(WCH):
        wj_k = big.tile([J, OCH, L], BF, tag=f"wj_{k}")
        nc.sync.dma_start(wj_k[:], wj_d[:, bass.ts(k, OCH), :])
        wj_tiles.append(wj_k)
    xb_sb = big.tile([128, B, C, L + 1], BF)
    nc.sync.dma_start(xb_sb[:], xb_d)
    xT_sb = big.tile([L, B, C, 128], BF)
    nc.sync.dma_start(xT_sb[:], xT_d)
    wt_sb = big.tile([L, O, J], BF)
    nc.sync.dma_start(wt_sb[:], wt_d)

    def wj_ap(o):
        return wj_tiles[o // OCH][:, o % OCH, :]

    # ---- constants ----
    ident_bf = const_pool.tile([128, 128], BF)
    make_identity(nc, ident_bf[:])
    ident_f = const_pool.tile([128, 128], F32)
    make_identity(nc, ident_f[:])
    ones_col128 = const_pool.tile([128, 1], BF)
    nc.vector.memset(ones_col128[:], 1.0)
    ones_row = const_pool.tile([1, L], BF)
    nc.vector.memset(ones_row[:], 1.0)
    ones_col64 = ones_col128[:L, :]

    # ---- G_o = W_o^T W_o, chunk-pipelined behind the w DMA ----
    GB = 8
    G_tiles = []
    for k in range(O // GB):
        g_ps = psL.tile([L, GB, L], F32, tag="lg")
        for i in range(GB):
            nc.tensor.matmul(g_ps[:, i, :], wj_ap(k * GB + i),
                             wj_ap(k * GB + i))
        G_k = big.tile([L, GB, L], BF, tag=f"G_{k}")
        if k % 2 == 0:
            nc.vector.tensor_copy(G_k[:], g_ps[:])
        else:
            nc.scalar.copy(G_k[:], g_ps[:])
        G_tiles.append(G_k)

    def G_ap(o):
        return G_tiles[o // GB][:, o % GB, :]

    # ---- u0[l, b] = sum_i x ----
    u0_ps = psQ.tile([L, B], F32, tag="q")
    for b in range(B):
        for c in range(C):
            nc.tensor.matmul(u0_ps[:, b : b + 1], xb_sb[:, b, c, :L],
                             ones_col128[:], start=(c == 0), stop=(c == C - 1))
    u0_sb = sbE.tile([L, B], BF, tag="u0")
    nc.vector.tensor_copy(u0_sb[:], u0_ps[:])

    def _dummy_out():
        nc.sync.dma_start(out_d[0, 0], ident_f[:1, :J])

    if probe == "P2":
        _dummy_out()
        return

    # ---- routing iterations; uTZ (l+Z, o, b) bf16 is the carried state ----
    uTZ_prev = None
    for t in range(1, NITER + 1):
        # p = G u  -> (l, o, b)
        pT_ps = psP.tile([L, O, B], F32, tag="pT")
        for o in range(O):
            rhs = u0_sb[:] if t == 1 else uTZ_prev[:L, o, :]
            nc.tensor.matmul(pT_ps[:, o, :], G_ap(o), rhs)
        pT_sb = sb.tile([L, O, B], BF, tag="pT_sb")
        nc.vector.tensor_copy(pT_sb[:], pT_ps[:])

        # q[o, b] = sum_l p*u via per-b column matmuls
        q_ps = psQ.tile([O, B], F32, tag="q")
        if t == 1:
            for b in range(B):
                nc.tensor.matmul(q_ps[:, b : b + 1], pT_sb[:, :, b],
                                 u0_sb[:, b : b + 1])
        else:
            qscr = sbE.tile([L, O, B], BF, tag="qscr")
            nc.vector.tensor_tensor(out=qscr[:], in0=pT_ps[:],
                                    in1=uTZ_prev[:L], op=ALU.mult)
            for b in range(B):
                nc.tensor.matmul(q_ps[:, b : b + 1], qscr[:, :, b],
                                 ones_col64)

        # rq = exp(-0.5 ln q) = 1/||W u||   (single ACT table)
        lnq = sbE.tile([O, B], F32, tag="lnq")
        nc.scalar.activation(lnq[:], q_ps[:], AF.Ln)
        rq = sbE.tile([O, B], BF, tag="rq")
        nc.scalar.activation(rq[:], lnq[:], AF.Exp, scale=-0.5)

        # broadcast rq over l: per-b transpose to a row, then ones-col matmul
        rqT_ps = psR.tile([1, B, O], BF, tag="rqT")
        for b in range(B):
            nc.tensor.transpose(rqT_ps[:, b, :], rq[:, b : b + 1],
                                ident_bf[:])
        rqT_sb = sbE.tile([1, B, O], BF, tag="rqTs")
        nc.vector.tensor_copy(rqT_sb[:], rqT_ps[:])
        rqb_ps = psB.tile([L, B, O], F32, tag="rqb")
        for b in range(B):
            nc.tensor.matmul(rqb_ps[:, b, :], ones_row[:],
                             rqT_sb[:, b, :])

        # v = p * rq  -> (l, o, b)
        v_sb = sb.tile([L, O, B], BF, tag="v")
        nc.vector.tensor_tensor(out=v_sb[:], in0=pT_sb[:],
                                in1=rqb_ps[:].transpose([0, 2, 1]),
                                op=ALU.mult)

        # logits -> exp -> u accumulation (Z rides along as row L)
        lg_tiles = []
        for b in range(B):
            lg_ps = psL.tile([128, C, O], F32, tag="lg")
            for c in range(C):
                nc.tensor.matmul(lg_ps[:, c, :], xT_sb[:, b, c, :],
                                 v_sb[:, :, b])
            lg_tiles.append(lg_ps)
        exp_tiles = []
        for b in range(B):
            exp_sb = sbE.tile([128, C, O], BF, tag=f"exp{b % 2}")
            nc.scalar.activation(exp_sb[:], lg_tiles[b][:], AF.Exp)
            exp_tiles.append(exp_sb)
        u_ps = psU.tile([L + 1, B, O], F32, tag="u")
        for b in range(B):
            for c in range(C):
                nc.tensor.matmul(u_ps[:, b, :], xb_sb[:, b, c, :],
                                 exp_tiles[b][:, c, :],
                                 start=(c == 0), stop=(c == C - 1))
        uTZ_sb = sb.tile([L + 1, O, B], BF, tag="uT")
        for b in range(B):
            nc.vector.tensor_copy(uTZ_sb[:, :, b], u_ps[:, b, :])

        if probe == f"I{t}":
            _dummy_out()
            return
        uTZ_prev = uTZ_sb

    # ---- rz = 1/Z ----
    z_ps = psR.tile([O, B, 2], BF, tag="rqT")
    for b in range(B):
        nc.tensor.transpose(z_ps[:, b, 0:1], uTZ_prev[L : L + 1, :, b],
                            ident_bf[L : L + 1, L : L + 1])
    rz_sb = sbE.tile([O, B], F32, tag="rz")
    nc.vector.reciprocal(rz_sb[:], z_ps[:, :, 0])

    # ---- out[b,o,:] = W_o u_3[o,:] / Z ----
    oT_ps = psP.tile([J, O, B], F32, tag="pT")
    for o in range(O):
        nc.tensor.matmul(oT_ps[:, o, :], wt_sb[:, o, :], uTZ_prev[:L, o, :])
    oT_sb = sb.tile([J, O, B], F32, tag="oT_sb")
    nc.scalar.copy(oT_sb[:, : O // 2, :], oT_ps[:, : O // 2, :])
    nc.vector.tensor_copy(oT_sb[:, O // 2 :, :], oT_ps[:, O // 2 :, :])
    out_all = sb.tile([O, B, J], F32, tag="out_sb")
    out_view = out_d.transpose([1, 0, 2])
    for b in range(B):
        o_ps = psB.tile([O, J], F32, tag="rqb")
        nc.tensor.transpose(o_ps[:], oT_sb[:, :, b], ident_f[:J, :J])
        nc.scalar.mul(out_all[:, b, :], o_ps[:], rz_sb[:, b : b + 1])
        if b % 2 == 1:
            nc.sync.dma_start(out_view[:, b - 1 : b + 1, :],
                              out_all[:, b - 1 : b + 1, :])


def build(probe=None):
    nc = bacc.Bacc("TRN2", target_bir_lowering=False, debug=False,
                   enable_asserts=True, num_devices=N_CORES)
    xb_d = nc.dram_tensor("xb", [128, B, C, L + 1], BF, kind="ExternalInput").ap()
    xT_d = nc.dram_tensor("xT", [L, B, C, 128], BF, kind="ExternalInput").ap()
    wj_d = nc.dram_tensor("wj", [J, O, L], BF, kind="ExternalInput").ap()
    wt_d = nc.dram_tensor("wt", [L, O, J], BF, kind="ExternalInput").ap()
    out_d = nc.dram_tensor("out", [B, O, J], F32, kind="ExternalOutput").ap()
    with tile.TileContext(nc) as tc:
        with ExitStack() as ctx:
            _body(ctx, tc, xb_d, xT_d, wj_d, wt_d, out_d, probe=probe)
    nc.compile()
    return nc


_NC = None
LAST_RESULTS = None


def _get_nc():
    global _NC
    if _NC is None:
        _NC = build()
    return _NC


def kernel(x: np.ndarray, weight: np.ndarray) -> np.ndarray:
    assert x.shape == (B_GLOBAL, I, L) and weight.shape == (O, J, L)
    nc = _get_nc()
    bf16 = ml_dtypes.bfloat16
    x = np.ascontiguousarray(x, dtype=np.float32)
    w = np.ascontiguousarray(weight, dtype=np.float32)
    wj = np.ascontiguousarray(w.transpose(1, 0, 2).astype(bf16))   # (j, o, l)
    wt = np.ascontiguousarray(w.transpose(2, 0, 1).astype(bf16))   # (l, o, j)
    in_maps = []
    for i in range(N_CORES):
        xs = x[i * B : (i + 1) * B]                  # (B, I, L)
        xr = xs.reshape(B, 128, C, L)                # i = 4p + c
        xb = np.empty((128, B, C, L + 1), dtype=bf16)
        xb[..., :L] = xr.transpose(1, 0, 2, 3).astype(bf16)
        xb[..., L] = 1.0
        xT = np.ascontiguousarray(xr.transpose(3, 0, 2, 1).astype(bf16))
        in_maps.append({"xb": xb, "xT": xT, "wj": wj, "wt": wt})
    global LAST_RESULTS
    LAST_RESULTS = bass_utils.run_bass_kernel_spmd(
        nc, in_maps, core_ids=list(range(N_CORES)))
    out = np.concatenate(
        [LAST_RESULTS.results[i]["out"] for i in range(N_CORES)], axis=0)
    return out.astype(np.float32)


# revision 11
# speedup vs baseline: 1.0322x; 1.0322x over previous
"""CapsuleLinear (k-means routing) Trainium2 kernel.

Math: priors[b,o,i,j] = sum_l w[o,j,l] x[b,i,l]; 3 rounds of k-means routing
over in_capsules, squash=False.

priors is never materialized.  With G_o = W_o^T W_o (64x64 per out-capsule,
computed on-device once):

    u_0[b,l]   = sum_i x[b,i,l]                  (scale of u is irrelevant)
    per iter:  p = G_o u;  q = u.p = ||W u||^2
               rq = exp(-0.5 ln q)  (= 1/||W u||, one ACT table: Ln/Exp/Copy)
               v = p * rq           (v = W^T out_normalized)
               logits[i,o] = sum_l x[b,i,l] v[o,l]
               e = exp(logits)      (softmax Z cancels in v)
               u[o,l] = sum_i e[i,o] x[b,i,l];  Z[o] = sum_i e[i,o]
    output:    out[b,o,:] = W_o u_3[o,:] / Z_3[o]

Sharding: data-parallel over batch, 4 samples/core x 8 cores, weight
replicated, no collectives.  Host passes pre-transposed bf16 layouts
(xb with a ones column, xT, w as (j,o,l) and (l,o,j)) so the device does
no dtype conversion or weight transposition.
"""

import sys

if "/opt/trn_rl_repo" not in sys.path:
    sys.path.insert(0, "/opt/trn_rl_repo")

from contextlib import ExitStack

import ml_dtypes
import numpy as np

import concourse.bacc as bacc
import concourse.bass as bass
import concourse.bass_utils as bass_utils
import concourse.mybir as mybir
import concourse.tile as tile
from concourse.masks import make_identity

BF = mybir.dt.bfloat16
F32 = mybir.dt.float32
AF = mybir.ActivationFunctionType
ALU = mybir.AluOpType

B_GLOBAL = 32
N_CORES = 8
B = B_GLOBAL // N_CORES  # 4 samples per core
O = 128   # out_capsules
I = 512   # in_capsules
J = 64    # out_length
L = 64    # in_length
C = 4     # i-chunks of 128
NITER = 3
WCH = 8   # w DMA chunks
OCH = O // WCH


def _body(ctx: ExitStack, tc: "tile.TileContext", xb_d, xT_d, wj_d, wt_d,
          out_d, probe=None):
    nc = tc.nc

    const_pool = ctx.enter_context(tc.tile_pool(name="const", bufs=1))
    big = ctx.enter_context(tc.tile_pool(name="big", bufs=1))
    sb = ctx.enter_context(tc.tile_pool(name="sb", bufs=2))
    sbE = ctx.enter_context(tc.tile_pool(name="sbE", bufs=3))
    psP = ctx.enter_context(tc.tile_pool(name="psP", bufs=2, space="PSUM"))
    psL = ctx.enter_context(tc.tile_pool(name="psL", bufs=2, space="PSUM"))
    psU = ctx.enter_context(tc.tile_pool(name="psU", bufs=1, space="PSUM"))
    psQ = ctx.enter_context(tc.tile_pool(name="psQ", bufs=1, space="PSUM"))
    psR = ctx.enter_context(tc.tile_pool(name="psR", bufs=1, space="PSUM"))
    psB = ctx.enter_context(tc.tile_pool(name="psB", bufs=1, space="PSUM"))

    # ---- input DMAs (w chunks first: G is the long head pole) ----
    wj_tiles = []
    for k in range(WCH):
        wj_k = big.tile([J, OCH, L], BF, tag=f"wj_{k}")
        nc.sync.dma_start(wj_k[:], wj_d[:, bass.ts(k, OCH), :])
        wj_tiles.append(wj_k)
    xb_sb = big.tile([128, B, C, L + 1], BF)
    nc.sync.dma_start(xb_sb[:], xb_d)
    xT_sb = big.tile([L, B, C, 128], BF)
    nc.sync.dma_start(xT_sb[:], xT_d)
    wt_sb = big.tile([L, O, J], BF)
    nc.sync.dma_start(wt_sb[:], wt_d)

    def wj_ap(o):
        return wj_tiles[o // OCH][:, o % OCH, :]

    # ---- constants ----
    ident_bf = const_pool.tile([128, 128], BF)
    make_identity(nc, ident_bf[:])
    ident_f = const_pool.tile([128, 128], F32)
    make_identity(nc, ident_f[:])
    ones_col128 = const_pool.tile([128, 1], BF)
    nc.vector.memset(ones_col128[:], 1.0)
    ones_row = const_pool.tile([1, L], BF)
    nc.vector.memset(ones_row[:], 1.0)
    ones_col64 = ones_col128[:L, :]

    # ---- G_o = W_o^T W_o, chunk-pipelined behind the w DMA ----
    GB = 8
    G_tiles = []
    for k in range(O // GB):
        g_ps = psL.tile([L, GB, L], F32, tag="lg")
        for i in range(GB):
            nc.tensor.matmul(g_ps[:, i, :], wj_ap(k * GB + i),
                             wj_ap(k * GB + i))
        G_k = big.tile([L, GB, L], BF, tag=f"G_{k}")
        if k % 2 == 0:
            nc.vector.tensor_copy(G_k[:], g_ps[:])
        else:
            nc.scalar.copy(G_k[:], g_ps[:])
        G_tiles.append(G_k)

    def G_ap(o):
        return G_tiles[o // GB][:, o % GB, :]

    # ---- u0[l, b] = sum_i x ----
    u0_ps = psQ.tile([L, B], F32, tag="q")
    for b in range(B):
        for c in range(C):
            nc.tensor.matmul(u0_ps[:, b : b + 1], xb_sb[:, b, c, :L],
                             ones_col128[:], start=(c == 0), stop=(c == C - 1))
    u0_sb = sbE.tile([L, B], BF, tag="u0")
    nc.vector.tensor_copy(u0_sb[:], u0_ps[:])

    def _dummy_out():
        nc.sync.dma_start(out_d[0, 0], ident_f[:1, :J])

    if probe == "P2":
        _dummy_out()
        return

    # ---- routing iterations; uTZ (l+Z, o, b) bf16 is the carried state ----
    uTZ_prev = None
    for t in range(1, NITER + 1):
        # p = G u  -> (l, o, b)
        pT_ps = psP.tile([L, O, B], F32, tag="pT")
        for o in range(O):
            rhs = u0_sb[:] if t == 1 else uTZ_prev[:L, o, :]
            nc.tensor.matmul(pT_ps[:, o, :], G_ap(o), rhs)
        pT_sb = sb.tile([L, O, B], BF, tag="pT_sb")
        nc.vector.tensor_copy(pT_sb[:], pT_ps[:])

        # q[o, b] = sum_l p*u via per-b column matmuls
        q_ps = psQ.tile([O, B], F32, tag="q")
        if t == 1:
            for b in range(B):
                nc.tensor.matmul(q_ps[:, b : b + 1], pT_sb[:, :, b],
                                 u0_sb[:, b : b + 1])
        else:
            qscr = sbE.tile([L, O, B], BF, tag="qscr")
            nc.vector.tensor_tensor(out=qscr[:], in0=pT_ps[:],
                                    in1=uTZ_prev[:L], op=ALU.mult)
            for b in range(B):
                nc.tensor.matmul(q_ps[:, b : b + 1], qscr[:, :, b],
                                 ones_col64)

        # rq = rsqrt(q) on DVE: bit-hack + 1 Newton step (keeps ACT on a
        # single activation table: only Exp/Copy are used there)
        I32 = mybir.dt.int32
        s_i = sbE.tile([O, B], I32, tag="rs_i")
        nc.vector.tensor_scalar(out=s_i[:], in0=q_ps[:].bitcast(I32),
                                scalar1=1, scalar2=None,
                                op0=ALU.arith_shift_right)
        y0_i = sbE.tile([O, B], I32, tag="rs_y0")
        nc.vector.tensor_scalar(out=y0_i[:], in0=s_i[:], scalar1=0x5F3759DF,
                                scalar2=-1, op0=ALU.subtract, op1=ALU.mult)
        y0f = y0_i[:].bitcast(F32)
        y2 = sbE.tile([O, B], F32, tag="rs_y2")
        nc.vector.tensor_tensor(out=y2[:], in0=y0f, in1=y0f, op=ALU.mult)
        t1 = sbE.tile([O, B], F32, tag="rs_t1")
        nc.vector.tensor_tensor(out=t1[:], in0=y2[:], in1=q_ps[:],
                                op=ALU.mult)
        t2 = sbE.tile([O, B], F32, tag="rs_t2")
        nc.vector.tensor_scalar(out=t2[:], in0=t1[:], scalar1=-0.5,
                                scalar2=1.5, op0=ALU.mult, op1=ALU.add)
        rq = sbE.tile([O, B], BF, tag="rq")
        nc.vector.tensor_tensor(out=rq[:], in0=y0f, in1=t2[:], op=ALU.mult)

        # broadcast rq over l: per-b transpose to a row, then ones-col matmul
        rqT_ps = psR.tile([1, B, O], BF, tag="rqT")
        for b in range(B):
            nc.tensor.transpose(rqT_ps[:, b, :], rq[:, b : b + 1],
                                ident_bf[:])
        rqT_sb = sbE.tile([1, B, O], BF, tag="rqTs")
        nc.vector.tensor_copy(rqT_sb[:], rqT_ps[:])
        rqb_ps = psB.tile([L, B, O], F32, tag="rqb")
        for b in range(B):
            nc.tensor.matmul(rqb_ps[:, b, :], ones_row[:],
                             rqT_sb[:, b, :])

        # v = p * rq  -> (l, o, b)
        v_sb = sb.tile([L, O, B], BF, tag="v")
        nc.vector.tensor_tensor(out=v_sb[:], in0=pT_sb[:],
                                in1=rqb_ps[:].transpose([0, 2, 1]),
                                op=ALU.mult)

        # logits -> exp -> u accumulation (Z rides along as row L)
        lg_tiles = []
        for b in range(B):
            lg_ps = psL.tile([128, C, O], F32, tag="lg")
            for c in range(C):
                nc.tensor.matmul(lg_ps[:, c, :], xT_sb[:, b, c, :],
                                 v_sb[:, :, b])
            lg_tiles.append(lg_ps)
        exp_tiles = []
        for b in range(B):
            exp_sb = sbE.tile([128, C, O], BF, tag=f"exp{b % 2}")
            nc.scalar.activation(exp_sb[:], lg_tiles[b][:], AF.Exp)
            exp_tiles.append(exp_sb)
        u_ps = psU.tile([L + 1, B, O], F32, tag="u")
        for b in range(B):
            for c in range(C):
                nc.tensor.matmul(u_ps[:, b, :], xb_sb[:, b, c, :],
                                 exp_tiles[b][:, c, :],
                                 start=(c == 0), stop=(c == C - 1))
        uTZ_sb = sb.tile([L + 1, O, B], BF, tag="uT")
        for b in range(B):
            nc.vector.tensor_copy(uTZ_sb[:, :, b], u_ps[:, b, :])

        if probe == f"I{t}":
            _dummy_out()
            return
        uTZ_prev = uTZ_sb

    # ---- rz = 1/Z ----
    z_ps = psR.tile([O, B, 2], BF, tag="rqT")
    for b in range(B):
        nc.tensor.transpose(z_ps[:, b, 0:1], uTZ_prev[L : L + 1, :, b],
                            ident_bf[L : L + 1, L : L + 1])
    rz_sb = sbE.tile([O, B], F32, tag="rz")
    nc.vector.reciprocal(rz_sb[:], z_ps[:, :, 0])

    # ---- out[b,o,:] = W_o u_3[o,:] / Z ----
    oT_ps = psP.tile([J, O, B], F32, tag="pT")
    for o in range(O):
        nc.tensor.matmul(oT_ps[:, o, :], wt_sb[:, o, :], uTZ_prev[:L, o, :])
    oT_sb = sb.tile([J, O, B], F32, tag="oT_sb")
    nc.scalar.copy(oT_sb[:, : O // 2, :], oT_ps[:, : O // 2, :])
    nc.vector.tensor_copy(oT_sb[:, O // 2 :, :], oT_ps[:, O // 2 :, :])
    out_all = sb.tile([O, B, J], F32, tag="out_sb")
    out_view = out_d.transpose([1, 0, 2])
    for b in range(B):
        o_ps = psB.tile([O, J], F32, tag="rqb")
        nc.tensor.transpose(o_ps[:], oT_sb[:, :, b], ident_f[:J, :J])
        nc.scalar.mul(out_all[:, b, :], o_ps[:], rz_sb[:, b : b + 1])
        if b % 2 == 1:
            nc.sync.dma_start(out_view[:, b - 1 : b + 1, :],
                              out_all[:, b - 1 : b + 1, :])


def build(probe=None):
    nc = bacc.Bacc("TRN2", target_bir_lowering=False, debug=False,
                   enable_asserts=True, num_devices=N_CORES)
    xb_d = nc.dram_tensor("xb", [128, B, C, L + 1], BF, kind="ExternalInput").ap()
    xT_d = nc.dram_tensor("xT", [L, B, C, 128], BF, kind="ExternalInput").ap()
    wj_d = nc.dram_tensor("wj", [J, O, L], BF, kind="ExternalInput").ap()
    wt_d = nc.dram_tensor("wt", [L, O, J], BF, kind="ExternalInput").ap()
    out_d = nc.dram_tensor("out", [B, O, J], F32, kind="ExternalOutput").ap()
    with tile.TileContext(nc) as tc:
        with ExitStack() as ctx:
            _body(ctx, tc, xb_d, xT_d, wj_d, wt_d, out_d, probe=probe)
    nc.compile()
    return nc


_NC = None
LAST_RESULTS = None


def _get_nc():
    global _NC
    if _NC is None:
        _NC = build()
    return _NC


def kernel(x: np.ndarray, weight: np.ndarray) -> np.ndarray:
    assert x.shape == (B_GLOBAL, I, L) and weight.shape == (O, J, L)
    nc = _get_nc()
    bf16 = ml_dtypes.bfloat16
    x = np.ascontiguousarray(x, dtype=np.float32)
    w = np.ascontiguousarray(weight, dtype=np.float32)
    wj = np.ascontiguousarray(w.transpose(1, 0, 2).astype(bf16))   # (j, o, l)
    wt = np.ascontiguousarray(w.transpose(2, 0, 1).astype(bf16))   # (l, o, j)
    in_maps = []
    for i in range(N_CORES):
        xs = x[i * B : (i + 1) * B]                  # (B, I, L)
        xr = xs.reshape(B, 128, C, L)                # i = 4p + c
        xb = np.empty((128, B, C, L + 1), dtype=bf16)
        xb[..., :L] = xr.transpose(1, 0, 2, 3).astype(bf16)
        xb[..., L] = 1.0
        xT = np.ascontiguousarray(xr.transpose(3, 0, 2, 1).astype(bf16))
        in_maps.append({"xb": xb, "xT": xT, "wj": wj, "wt": wt})
    global LAST_RESULTS
    LAST_RESULTS = bass_utils.run_bass_kernel_spmd(
        nc, in_maps, core_ids=list(range(N_CORES)))
    out = np.concatenate(
        [LAST_RESULTS.results[i]["out"] for i in range(N_CORES)], axis=0)
    return out.astype(np.float32)


# revision 16
# speedup vs baseline: 1.2922x; 1.2519x over previous
"""CapsuleLinear (k-means routing) Trainium2 kernel.

Math: priors[b,o,i,j] = sum_l w[o,j,l] x[b,i,l]; 3 rounds of k-means routing
over in_capsules, squash=False.

priors is never materialized.  With G_o = W_o^T W_o (64x64 per out-capsule,
computed on-device once):

    u_0[b,l]   = sum_i x[b,i,l]                  (scale of u is irrelevant)
    per iter:  p = G_o u;  q = u.p = ||W u||^2
               rq = exp(-0.5 ln q)  (= 1/||W u||, one ACT table: Ln/Exp/Copy)
               v = p * rq           (v = W^T out_normalized)
               logits[i,o] = sum_l x[b,i,l] v[o,l]
               e = exp(logits)      (softmax Z cancels in v)
               u[o,l] = sum_i e[i,o] x[b,i,l];  Z[o] = sum_i e[i,o]
    output:    out[b,o,:] = W_o u_3[o,:] / Z_3[o]

Sharding: data-parallel over batch, 4 samples/core x 8 cores, weight
replicated, no collectives.  Host passes pre-transposed bf16 layouts
(xb with a ones column, xT, w as (j,o,l) and (l,o,j)) so the device does
no dtype conversion or weight transposition.
"""

import sys

if "/opt/trn_rl_repo" not in sys.path:
    sys.path.insert(0, "/opt/trn_rl_repo")

from contextlib import ExitStack

import ml_dtypes
import numpy as np

import concourse.bacc as bacc
import concourse.bass as bass
import concourse.bass_utils as bass_utils
import concourse.mybir as mybir
import concourse.tile as tile
from concourse.masks import make_identity

BF = mybir.dt.bfloat16
F32 = mybir.dt.float32
AF = mybir.ActivationFunctionType
ALU = mybir.AluOpType

B_GLOBAL = 32
N_CORES = 8
B = B_GLOBAL // N_CORES  # 4 samples per core
O = 128   # out_capsules
I = 512   # in_capsules
J = 64    # out_length
L = 64    # in_length
C = 4     # i-chunks of 128
NITER = 3
WCH = 8   # w DMA chunks
OCH = O // WCH


def _body(ctx: ExitStack, tc: "tile.TileContext", xb_d, xT_d, wj_d, wt_d,
          out_d, probe=None):
    nc = tc.nc

    const_pool = ctx.enter_context(tc.tile_pool(name="const", bufs=1))
    big = ctx.enter_context(tc.tile_pool(name="big", bufs=1))
    sb = ctx.enter_context(tc.tile_pool(name="sb", bufs=2))
    sbE = ctx.enter_context(tc.tile_pool(name="sbE", bufs=3))
    psP = ctx.enter_context(tc.tile_pool(name="psP", bufs=1, space="PSUM"))
    psL = ctx.enter_context(tc.tile_pool(name="psL", bufs=2, space="PSUM"))
    psU = ctx.enter_context(tc.tile_pool(name="psU", bufs=2, space="PSUM"))
    psQ = ctx.enter_context(tc.tile_pool(name="psQ", bufs=1, space="PSUM"))
    psR = ctx.enter_context(tc.tile_pool(name="psR", bufs=1, space="PSUM"))
    psB = ctx.enter_context(tc.tile_pool(name="psB", bufs=1, space="PSUM"))

    # ---- input DMAs (w chunks first: G is the long head pole) ----
    wj_tiles = []
    for k in range(WCH):
        wj_k = big.tile([J, OCH, L], BF, tag=f"wj_{k}")
        nc.sync.dma_start(wj_k[:], wj_d[:, bass.ts(k, OCH), :])
        wj_tiles.append(wj_k)
    xb_sb = big.tile([128, B, C, L + 1], BF)
    nc.sync.dma_start(xb_sb[:], xb_d)
    xT_sb = big.tile([L, B, C, 128], BF)
    nc.sync.dma_start(xT_sb[:], xT_d)
    wt_sb = big.tile([L, O, J], BF)
    nc.sync.dma_start(wt_sb[:], wt_d)

    def wj_ap(o):
        return wj_tiles[o // OCH][:, o % OCH, :]

    # ---- constants ----
    ident_bf = const_pool.tile([128, 128], BF)
    make_identity(nc, ident_bf[:])
    ident_f = const_pool.tile([128, 128], F32)
    make_identity(nc, ident_f[:])
    ones_col128 = const_pool.tile([128, 1], BF)
    nc.vector.memset(ones_col128[:], 1.0)
    ones_row = const_pool.tile([1, L], BF)
    nc.vector.memset(ones_row[:], 1.0)
    ones_col64 = ones_col128[:L, :]

    # ---- G_o = W_o^T W_o, chunk-pipelined behind the w DMA ----
    GB = 8
    G_tiles = []
    for k in range(O // GB):
        g_ps = psL.tile([L, GB, L], F32, tag="lg")
        for i in range(GB):
            nc.tensor.matmul(g_ps[:, i, :], wj_ap(k * GB + i),
                             wj_ap(k * GB + i))
        G_k = big.tile([L, GB, L], BF, tag=f"G_{k}")
        if k % 2 == 0:
            nc.vector.tensor_copy(G_k[:], g_ps[:])
        else:
            nc.scalar.copy(G_k[:], g_ps[:])
        G_tiles.append(G_k)

    def G_ap(o):
        return G_tiles[o // GB][:, o % GB, :]

    # ---- u0[l, b] = sum_i x ----
    u0_ps = psQ.tile([L, B], F32, tag="q")
    for b in range(B):
        for c in range(C):
            nc.tensor.matmul(u0_ps[:, b : b + 1], xb_sb[:, b, c, :L],
                             ones_col128[:], start=(c == 0), stop=(c == C - 1))
    u0_sb = sbE.tile([L, B], BF, tag="u0")
    nc.vector.tensor_copy(u0_sb[:], u0_ps[:])

    def _dummy_out():
        nc.sync.dma_start(out_d[0, 0], ident_f[:1, :J])

    if probe == "P2":
        _dummy_out()
        return

    # ---- routing iterations; uTZ (l+Z, o, b) bf16 is the carried state ----
    uTZ_prev = None
    for t in range(1, NITER + 1):
        # p = G u  -> (l, o, b)
        pT_ps = psP.tile([L, O, B], F32, tag="pT")
        for o in range(O):
            rhs = u0_sb[:] if t == 1 else uTZ_prev[:L, o, :]
            nc.tensor.matmul(pT_ps[:, o, :], G_ap(o), rhs)
        # q[o, b] = sum_l p*u: elementwise against PSUM, then per-b column
        # matmuls.  Emitted before the pT copy so the q chain leads the DVE
        # queue; the copy drains later, off the critical path.
        qscr = sbE.tile([L, O, B], BF, tag="qscr")
        u_in = (u0_sb[:].unsqueeze(1).broadcast_to([L, O, B]) if t == 1
                else uTZ_prev[:L])
        nc.vector.tensor_tensor(out=qscr[:], in0=pT_ps[:], in1=u_in,
                                op=ALU.mult)
        q_ps = psQ.tile([O, B], F32, tag="q")
        for b in range(B):
            nc.tensor.matmul(q_ps[:, b : b + 1], qscr[:, :, b], ones_col64)

        # rq = rsqrt(q) on DVE: bit-hack + 1 Newton step, sign folded so the
        # chain is 5 ops (keeps ACT on one table: only Exp/Copy used there)
        I32 = mybir.dt.int32
        s_i = sbE.tile([O, B], I32, tag="rs_s")
        nc.vector.tensor_scalar(out=s_i[:], in0=q_ps[:].bitcast(I32),
                                scalar1=1, scalar2=None,
                                op0=ALU.arith_shift_right)
        y0_i = sbE.tile([O, B], I32, tag="rs_y0")
        nc.vector.tensor_scalar(out=y0_i[:], in0=s_i[:], scalar1=0x5F3759DF,
                                scalar2=-1, op0=ALU.subtract, op1=ALU.mult)
        y0f = y0_i[:].bitcast(F32)
        y2 = sbE.tile([O, B], F32, tag="rs_y2")
        nc.vector.tensor_tensor(out=y2[:], in0=y0f, in1=y0f, op=ALU.mult)
        t1 = sbE.tile([O, B], F32, tag="rs_t1")
        nc.vector.tensor_tensor(out=t1[:], in0=y2[:], in1=q_ps[:],
                                op=ALU.mult)
        t2 = sbE.tile([O, B], F32, tag="rs_t2")
        nc.vector.tensor_scalar(out=t2[:], in0=t1[:], scalar1=-0.5,
                                scalar2=1.5, op0=ALU.mult, op1=ALU.add)
        rq = sbE.tile([O, B], BF, tag="rq")
        nc.vector.tensor_tensor(out=rq[:], in0=y0f, in1=t2[:], op=ALU.mult)

        # pT -> SBUF (needed by the v product, not by the q chain)
        pT_sb = sb.tile([L, O, B], BF, tag="pT_sb")
        nc.vector.tensor_copy(pT_sb[:], pT_ps[:])

        # broadcast rq over l: per-b transpose to a row, then ones-col matmul
        rqT_ps = psR.tile([1, B, O], BF, tag="rqT")
        for b in range(B):
            nc.tensor.transpose(rqT_ps[:, b, :], rq[:, b : b + 1],
                                ident_bf[:])
        rqT_sb = sbE.tile([1, B, O], BF, tag="rqTs")
        nc.vector.tensor_copy(rqT_sb[:], rqT_ps[:])
        rqb_ps = psB.tile([L, B, O], F32, tag="rqb")
        for b in range(B):
            nc.tensor.matmul(rqb_ps[:, b, :], ones_row[:],
                             rqT_sb[:, b, :])

        # v = p * rq  -> (l, o, b)
        v_sb = sb.tile([L, O, B], BF, tag="v")
        nc.vector.tensor_tensor(out=v_sb[:], in0=pT_sb[:],
                                in1=rqb_ps[:].transpose([0, 2, 1]),
                                op=ALU.mult)

        # logits -> exp -> u accumulation (Z rides along as row L)
        lg_tiles = []
        for b in range(B):
            lg_ps = psL.tile([128, C, O], F32, tag="lg")
            for c in range(C):
                nc.tensor.matmul(lg_ps[:, c, :], xT_sb[:, b, c, :],
                                 v_sb[:, :, b])
            lg_tiles.append(lg_ps)
        exp_tiles = []
        for b in range(B):
            exp_sb = sbE.tile([128, C, O], BF, tag=f"exp{b % 2}")
            nc.scalar.activation(exp_sb[:], lg_tiles[b][:], AF.Exp)
            exp_tiles.append(exp_sb)
        # per-pair u tiles so each uTZ copy fires as soon as its own ua
        # matmuls finish (tile-granular dependency tracking)
        uTZ_sb = sb.tile([L + 1, O, B], BF, tag="uT")
        for pair in range(2):
            u_ps = psU.tile([L + 1, 2, O], F32, tag="u")
            for half in range(2):
                b = pair * 2 + half
                for c in range(C):
                    nc.tensor.matmul(u_ps[:, half, :], xb_sb[:, b, c, :],
                                     exp_tiles[b][:, c, :],
                                     start=(c == 0), stop=(c == C - 1))
            nc.vector.tensor_copy(
                uTZ_sb[:, :, pair * 2 : pair * 2 + 2],
                u_ps[:].transpose([0, 2, 1]))

        if probe == f"I{t}":
            _dummy_out()
            return
        uTZ_prev = uTZ_sb

    # ---- rz = 1/Z ----
    z_ps = psR.tile([O, B, 2], BF, tag="rqT")
    for b in range(B):
        nc.tensor.transpose(z_ps[:, b, 0:1], uTZ_prev[L : L + 1, :, b],
                            ident_bf[L : L + 1, L : L + 1])
    rz_sb = sbE.tile([O, B], F32, tag="rz")
    nc.vector.reciprocal(rz_sb[:], z_ps[:, :, 0])

    # ---- out[b,o,:] = W_o u_3[o,:] / Z ----
    oT_ps = psP.tile([J, O, B], F32, tag="pT")
    for o in range(O):
        nc.tensor.matmul(oT_ps[:, o, :], wt_sb[:, o, :], uTZ_prev[:L, o, :])
    oT_sb = sb.tile([J, O, B], F32, tag="oT_sb")
    nc.scalar.copy(oT_sb[:, : O // 2, :], oT_ps[:, : O // 2, :])
    nc.vector.tensor_copy(oT_sb[:, O // 2 :, :], oT_ps[:, O // 2 :, :])
    out_all = sb.tile([O, B, J], F32, tag="out_sb")
    out_view = out_d.transpose([1, 0, 2])
    for b in range(B):
        o_ps = psB.tile([O, J], F32, tag="rqb")
        nc.tensor.transpose(o_ps[:], oT_sb[:, :, b], ident_f[:J, :J])
        nc.scalar.mul(out_all[:, b, :], o_ps[:], rz_sb[:, b : b + 1])
        if b % 2 == 1:
            nc.sync.dma_start(out_view[:, b - 1 : b + 1, :],
                              out_all[:, b - 1 : b + 1, :])


def build(probe=None):
    nc = bacc.Bacc("TRN2", target_bir_lowering=False, debug=False,
                   enable_asserts=True, num_devices=N_CORES)
    xb_d = nc.dram_tensor("xb", [128, B, C, L + 1], BF, kind="ExternalInput").ap()
    xT_d = nc.dram_tensor("xT", [L, B, C, 128], BF, kind="ExternalInput").ap()
    wj_d = nc.dram_tensor("wj", [J, O, L], BF, kind="ExternalInput").ap()
    wt_d = nc.dram_tensor("wt", [L, O, J], BF, kind="ExternalInput").ap()
    out_d = nc.dram_tensor("out", [B, O, J], F32, kind="ExternalOutput").ap()
    with tile.TileContext(nc) as tc:
        with ExitStack() as ctx:
            _body(ctx, tc, xb_d, xT_d, wj_d, wt_d, out_d, probe=probe)
    nc.compile()
    return nc


_NC = None
LAST_RESULTS = None


def _get_nc():
    global _NC
    if _NC is None:
        _NC = build()
    return _NC


def kernel(x: np.ndarray, weight: np.ndarray) -> np.ndarray:
    assert x.shape == (B_GLOBAL, I, L) and weight.shape == (O, J, L)
    nc = _get_nc()
    bf16 = ml_dtypes.bfloat16
    x = np.ascontiguousarray(x, dtype=np.float32)
    w = np.ascontiguousarray(weight, dtype=np.float32)
    wj = np.ascontiguousarray(w.transpose(1, 0, 2).astype(bf16))   # (j, o, l)
    wt = np.ascontiguousarray(w.transpose(2, 0, 1).astype(bf16))   # (l, o, j)
    in_maps = []
    for i in range(N_CORES):
        xs = x[i * B : (i + 1) * B]                  # (B, I, L)
        xr = xs.reshape(B, 128, C, L)                # i = 4p + c
        xb = np.empty((128, B, C, L + 1), dtype=bf16)
        xb[..., :L] = xr.transpose(1, 0, 2, 3).astype(bf16)
        xb[..., L] = 1.0
        xT = np.ascontiguousarray(xr.transpose(3, 0, 2, 1).astype(bf16))
        in_maps.append({"xb": xb, "xT": xT, "wj": wj, "wt": wt})
    global LAST_RESULTS
    LAST_RESULTS = bass_utils.run_bass_kernel_spmd(
        nc, in_maps, core_ids=list(range(N_CORES)))
    out = np.concatenate(
        [LAST_RESULTS.results[i]["out"] for i in range(N_CORES)], axis=0)
    return out.astype(np.float32)
